# revision 1
# baseline (speedup 1.0000x reference)
"""Trainium2 Bass kernel for CropSplit (SipMask-style crop + quadrant split).

Reference computation, per output pixel (y, x, n):
    inside = point (x, y) lies in box rois[n] = (x1, y1, x2, y2)
    cell   = which of the 2x2 ROI sub-cells the pixel falls in
    out[y, x, n] = inside ? data[cell, y, x, n] : 0

Strategy (v6 — DMA-engine balanced, masks generated on device):
  - Shard along W across the 8 cores (25 columns each); tile layout
    [h -> partitions, (cc, w, n) -> free] so every DMA row is a large
    contiguous DRAM block.
  - Each of the 16 SDMA engines serves a fixed group of 8 SBUF
    partitions, so per-engine bytes are set by how rows map to
    partitions.  H=200 rows are mapped:
      rows   0..127 -> partitions p = h          (stride 1: 8 rows/group)
      rows 128..191 -> partitions p = 2(h-128)   (stride 2: 4 rows/group)
      rows 192..199 -> one flat mini-tile [100 partitions x 2 (h,w)-runs]
    => every engine group moves ~12.5 rows instead of the 8/16 split a
    naive 128+72 chunking gives (that imbalance capped the old kernel at
    ~75% DMA utilization in its second half).
  - Only tiny per-(w,n) / per-(h,n) mask factors are shipped (u8):
      hx[w,n], nix[w,n]   one partition's copy (10 KB), gpsimd
                          partition_broadcast fans them out per w-block
      hy/niy packed [p, hpart, n] for the two row layouts (100 KB)
      per-element masks only for the tiny 8-row mini-tile (120 KB)
    The big per-pixel not-inside mask (1.28 MB/core in the shipped
    version) is generated on the idle gpsimd engine per w-block:
      nin[p, hpart, w, n] = max(nix[w, n], niy[p, hpart, n])
    which cuts HBM traffic by ~6%.
  - Per tile, the 4-way select + mask is 3 predicated DVE ops, ordered
    y-blend -> x-blend -> zero so the first blocks' compute only needs
    the y-masks (first small DMA) while gpsimd works through the
    broadcasts:
        cp(dall[0:2], hy, dall[2:4])  (d0<-d2, d1<-d3 where cy==1)
        cp(dall[0], hx, dall[1])      (x-blend -> 4-way select)
        cp(dall[0], nin, 0)           (zero outside the box)
  - w-blocks [3,6,6,6,3,1]: small first (fast ramp), small last (short
    drain tail).  The mini-tile goes first of all: its store primes the
    store path while the big loads stream.
  - DMA issue is split across both HWDGE sequencers (Sync for data
    loads, Scalar for masks/stores).
"""

import numpy as np

C = 2
CC = C * C
H = W = N = 200
NCORES = 8
WS = W // NCORES  # 25 columns per core

# w-blocks: (w0, wb). Small first (fast pipeline ramp), small last
# (short drain tail).
W_BLOCKS = [(0, 3), (3, 6), (9, 6), (15, 6), (21, 3), (24, 1)]
DATA_BUFS = 6

_cache: dict = {}


def _rowmap():
    """rm[hpart][p] = source row h for partition p (or -1 = unused)."""
    m = np.full((2, 128), -1, dtype=np.int64)
    m[0, :] = np.arange(128)
    m[1, 0:128:2] = 128 + np.arange(64)
    return m


_ROWMAP = _rowmap()
# mini-tile: partition p holds (h,w)-runs f=2p, 2p+1 of the 8x25 slab
_MB_F = np.arange(200).reshape(100, 2)
_MB_H = 192 + _MB_F // WS
_MB_W = _MB_F % WS  # column index within the core's 25


def _build_module():
    import concourse.bacc as bacc
    import concourse.mybir as mybir
    from concourse.tile import TileContext

    f32 = mybir.dt.float32
    u8 = mybir.dt.uint8
    i32 = mybir.dt.int32

    nc = bacc.Bacc(trn_type="TRN2", debug=False, num_devices=NCORES)
    data = nc.dram_tensor("data", [CC, H, WS, N], f32, kind="ExternalInput")
    # hx / nix for this core's 25 columns: one partition's copy.
    # nix is f32 because the gpsimd nin-combine must be a float op.
    mx1 = nc.dram_tensor("mx1", [1, WS, N], u8, kind="ExternalInput")
    nix1 = nc.dram_tensor("nix1", [1, WS, N], i32, kind="ExternalInput")
    # hy packed per hpart row layout: [p, hpart, n]; niy same but f32
    yb = nc.dram_tensor("yb", [128, 2, N], u8, kind="ExternalInput")
    niyb = nc.dram_tensor("niyb", [128, 2, N], i32, kind="ExternalInput")
    # mini-tile (rows 192..199) per-element masks: [p, {hx,hy,nin}, run, n]
    mb = nc.dram_tensor("mb", [128, 3, 2, N], u8, kind="ExternalInput")
    out = nc.dram_tensor("out", [H, WS, N], f32, kind="ExternalOutput")

    with TileContext(nc) as tc:
        with (
            tc.tile_pool(name="masks", bufs=1) as mpool,
            tc.tile_pool(name="dpool", bufs=DATA_BUFS) as dpool,
            tc.tile_pool(name="npool", bufs=3) as npool,
        ):
            zeros = mpool.tile([128, 1], f32)
            nc.vector.memset(zeros[:], 0.0)

            # small mask loads first (scalar queue)
            ymask = mpool.tile([128, 2, N], u8, tag="yb")
            nc.scalar.dma_start(ymask[:], yb[:])
            niyt = mpool.tile([128, 2, N], i32, tag="niyb")
            nc.scalar.dma_start(niyt[:], niyb[:])
            mbm = mpool.tile([128, 3, 2, N], u8, tag="mb")
            nc.scalar.dma_start(mbm[:], mb[:])
            mx_row = mpool.tile([1, WS, N], u8, tag="mxrow")
            nc.sync.dma_start(mx_row[:], mx1[:])
            nix_row = mpool.tile([1, WS, N], i32, tag="nixrow")
            nc.sync.dma_start(nix_row[:], nix1[:])
            mxt = mpool.tile([128, WS, N], u8, tag="mxb")
            nxt = mpool.tile([128, WS, N], i32, tag="nxb")

            # ---- mini-tile: rows 192..199 as [100, cc, 2, N] ----
            dmini = dpool.tile([128, CC, 2, N], f32, tag="dall")
            src_mb = (
                data[:, 192:200, :, :]
                .rearrange("c h w n -> c (h w) n")
                .rearrange("c (p r) n -> p c r n", p=100)
            )
            nc.sync.dma_start(dmini[0:100], src_mb)
            zv2 = zeros[:, :, None].broadcast_to((128, 2, N))
            nc.vector.copy_predicated(
                dmini[:, 0:2],
                mbm[:, 1, None, :, :].broadcast_to((128, 2, 2, N)),
                dmini[:, 2:4],
            )
            nc.vector.copy_predicated(dmini[:, 0], mbm[:, 0], dmini[:, 1])
            nc.vector.copy_predicated(dmini[:, 0], mbm[:, 2], zv2)
            nc.scalar.dma_start(
                out[192:200, :, :]
                .rearrange("h w n -> (h w) n")
                .rearrange("(p r) n -> p r n", p=100),
                dmini[0:100, 0],
            )

            # ---- main blocks ----
            for bi, (w0, wb) in enumerate(W_BLOCKS):
                # per-block gpsimd work: broadcast hx/nix columns, then
                # build nin = max(nix, niy) for both hparts at once.
                nc.gpsimd.partition_broadcast(
                    mxt[:, w0 : w0 + wb, :], mx_row[:, w0 : w0 + wb, :]
                )
                nc.gpsimd.partition_broadcast(
                    nxt[:, w0 : w0 + wb, :], nix_row[:, w0 : w0 + wb, :]
                )
                # all-int32: Pool integer ops require matching 32-bit
                # dtypes; int32 is also a legal copy_predicated mask type.
                t_nin = npool.tile([128, 2, wb, N], i32, tag="nin")
                nc.gpsimd.tensor_tensor(
                    t_nin[:],
                    nxt[:, None, w0 : w0 + wb, :].broadcast_to(
                        (128, 2, wb, N)
                    ),
                    niyt[:, :, None, :].broadcast_to((128, 2, wb, N)),
                    mybir.AluOpType.add,
                )
                for hp in range(2):
                    dall = dpool.tile([128, CC, wb, N], f32, tag="dall")
                    src = data[:, :, w0 : w0 + wb, :]
                    if hp == 0:
                        nc.sync.dma_start(
                            dall[:], src[:, 0:128].transpose([1, 0, 2, 3])
                        )
                    else:
                        nc.sync.dma_start(
                            dall[0:128:2],
                            src[:, 128:192].transpose([1, 0, 2, 3]),
                        )
                    hyv2 = ymask[:, hp, None, None, :].broadcast_to(
                        (128, 2, wb, N)
                    )
                    zv = zeros[:, :, None].broadcast_to((128, wb, N))
                    # y-blend both cell columns in one op, then x-blend,
                    # then zero.
                    nc.vector.copy_predicated(
                        dall[:, 0:2], hyv2, dall[:, 2:4]
                    )
                    nc.vector.copy_predicated(
                        dall[:, 0], mxt[:, w0 : w0 + wb, :], dall[:, 1]
                    )
                    nc.vector.copy_predicated(
                        dall[:, 0], t_nin[:, hp], zv
                    )
                    if hp == 0:
                        nc.scalar.dma_start(
                            out[0:128, w0 : w0 + wb, :], dall[0:128, 0]
                        )
                    else:
                        nc.scalar.dma_start(
                            out[128:192, w0 : w0 + wb, :], dall[0:128:2, 0]
                        )
    nc.finalize()
    return nc


def _get_module():
    if "nc" not in _cache:
        _cache["nc"] = _build_module()
    return _cache["nc"]


def _host_masks(rois):
    """Masks in f32 arithmetic bit-identical to the reference, as uint8."""
    r = np.asarray(rois, dtype=np.float32)
    x1, y1, x2, y2 = r[:, 0], r[:, 1], r[:, 2], r[:, 3]
    two = np.float32(2.0)
    one = np.float32(1.0)

    xs = np.arange(W, dtype=np.float32)[:, None]  # (W, 1)
    cw = np.maximum(x2 - x1, one)[None, :]  # (1, N)
    fx = np.floor(two * (xs - x1[None, :]) / cw)
    hx = (fx >= 1.0).astype(np.uint8)  # clip(floor, 0, 1) == 1
    nix = (~((xs >= x1[None, :]) & (xs <= x2[None, :]))).astype(np.uint8)

    ys = np.arange(H, dtype=np.float32)[:, None]  # (H, 1)
    ch = np.maximum(y2 - y1, one)[None, :]
    fy = np.floor(two * (ys - y1[None, :]) / ch)
    hy = (fy >= 1.0).astype(np.uint8)
    niy = (~((ys >= y1[None, :]) & (ys <= y2[None, :]))).astype(np.uint8)

    return hx, nix, hy, niy


def _pack_rows(arr_by_row, fill):
    """arr_by_row: (H, ...) -> packed (128, 2, ...) per hpart."""
    shp = (128, 2) + arr_by_row.shape[1:]
    outp = np.full(shp, fill, dtype=arr_by_row.dtype)
    for hp in range(2):
        valid = _ROWMAP[hp] >= 0
        outp[valid, hp] = arr_by_row[_ROWMAP[hp][valid]]
    return outp


def _run(data, rois, trace=False):
    from concourse.bass_utils import run_bass_kernel_spmd

    data = np.ascontiguousarray(np.asarray(data, dtype=np.float32))
    hx, nix, hy, niy = _host_masks(rois)

    # hy/niy packed per hpart: [128, 2, N]
    yb = np.ascontiguousarray(_pack_rows(hy, 0))
    niyb = np.ascontiguousarray(_pack_rows(niy, 1).astype(np.int32))

    in_maps = []
    for i in range(NCORES):
        sl = slice(i * WS, (i + 1) * WS)
        hx_c = hx[sl, :]  # (WS, N)
        nix_c = nix[sl, :]
        # mini-tile per-element masks [128, 3, 2, N]
        mbq = np.ones((128, 3, 2, N), dtype=np.uint8)
        mbq[:100, 0] = hx_c[_MB_W]  # hx
        mbq[:100, 1] = hy[_MB_H]  # hy
        mbq[:100, 2] = np.maximum(nix_c[_MB_W], niy[_MB_H])  # nin
        in_maps.append(
            {
                "data": np.ascontiguousarray(data[:, :, sl, :]),
                "mx1": np.ascontiguousarray(hx_c[None]),
                "nix1": np.ascontiguousarray(
                    nix_c[None].astype(np.int32)
                ),
                "yb": yb,
                "niyb": niyb,
                "mb": np.ascontiguousarray(mbq),
            }
        )

    nc = _get_module()
    last_err = None
    for _attempt in range(2):
        try:
            res = run_bass_kernel_spmd(
                nc, in_maps, core_ids=list(range(NCORES)), trace=trace
            )
            break
        except Exception as e:  # transient NRT device errors: retry once
            last_err = e
    else:
        raise last_err
    full = np.concatenate([r["out"] for r in res.results], axis=1)
    return np.asarray(full, dtype=np.float32), res


def kernel(data, rois):
    out, _ = _run(data, rois, trace=False)
    return out



# revision 2
# speedup vs baseline: 1.0201x; 1.0201x over previous
"""Trainium2 Bass kernel for CropSplit (SipMask-style crop + quadrant split).

Reference computation, per output pixel (y, x, n):
    inside = point (x, y) lies in box rois[n] = (x1, y1, x2, y2)
    cell   = which of the 2x2 ROI sub-cells the pixel falls in
    out[y, x, n] = inside ? data[cell, y, x, n] : 0

Strategy (v9 — bf16, column-major fully-packed single pass):
  - bf16 end-to-end (gate is rel_err < 2e-2; bf16 lands ~3e-3): halves
    HBM traffic, and tensor_tensor bf16 hits the 2x_1p DVE perf mode.
  - Shard along W across the 8 cores (25 columns each).  Flatten the
    core's plane COLUMN-major: f = w*H + h, pack f = 40p + r
    (125 partitions x 40 runs, padded to 128).  Each partition then
    covers exactly ONE column w = p//5 (rows 40*(p%5) .. +39), so the
    x-masks are constant per partition: a tiny resident [128, N] pred
    broadcast over runs (fast 1:rb pred:data ratio) instead of a
    1 MB per-element tensor (v8).  y-mask stays per-element u8; the
    combined inside-mask (bf16, for the 2x TT multiply) folds both
    crop conditions into one op.
  - Per run-block, 3 DVE ops:
      cp(dall[0:2], hye, dall[2:4])   y-blend   (2*FD @ ~1.08 cyc/el)
      cp(dall[0],  hxp, dall[1])      x-blend   (FD, broadcast pred)
      dall[0] *= nin                  zero outside (FD @ 2x)
  - Each (c, p) data source block is 40*200 contiguous bf16 = 16 KB.
    Data loads go as one 2D [128, rb*N] DMA per channel (4D APs with
    the big c-stride mid-pattern unbalance the 16 SDMA engines).
  - Queues: Sync = data; Scalar = masks + stores (issue overlap).
"""

import numpy as np
import ml_dtypes

BF16 = ml_dtypes.bfloat16

C = 2
CC = C * C
H = W = N = 200
NCORES = 8
WS = W // NCORES  # 25 columns per core
F = H * WS  # 5000 flattened (w, h) columns per core
P = 128  # partitions (F padded to P * R with zero columns)
R = 40  # runs per partition
FP = P * R  # 5120 padded columns
PV = F // R  # 125 real partitions

RB_BLOCKS = [(0, 3), (3, 6), (9, 8), (17, 8), (25, 8), (33, 7)]
DATA_BUFS = 8

_cache: dict = {}

# partition/run -> (w_local, h); valid for p < PV
_P_IDX = np.arange(PV)
_P_W = _P_IDX // 5  # column per partition
_PR_H = (40 * (_P_IDX % 5))[:, None] + np.arange(R)[None, :]  # [PV, R]


def _build_module():
    import concourse.bacc as bacc
    import concourse.mybir as mybir
    from concourse.tile import TileContext

    bf16 = mybir.dt.bfloat16
    u8 = mybir.dt.uint8
    mult = mybir.AluOpType.mult

    nc = bacc.Bacc(trn_type="TRN2", debug=False, num_devices=NCORES)
    data = nc.dram_tensor("data", [CC, FP, N], bf16, kind="ExternalInput")
    hxp = nc.dram_tensor("hxp", [128, N], u8, kind="ExternalInput")
    hye = nc.dram_tensor("hye", [128, R, N], u8, kind="ExternalInput")
    nin = nc.dram_tensor("nin", [128, R, N], bf16, kind="ExternalInput")
    out = nc.dram_tensor("out", [FP, N], bf16, kind="ExternalOutput")

    data_r = data.rearrange("c (p r) n -> p c r n", p=P)
    out_r = out.rearrange("(p r) n -> p r n", p=P)

    with TileContext(nc) as tc:
        with (
            tc.tile_pool(name="mpool", bufs=4) as mpool,
            tc.tile_pool(name="spool", bufs=1) as spool,
            tc.tile_pool(name="dpool", bufs=DATA_BUFS) as dpool,
        ):
            hxt = spool.tile([128, N], u8, tag="hxp")
            nc.scalar.dma_start(hxt[:], hxp[:])
            for r0, rb in RB_BLOCKS:
                sl = slice(r0, r0 + rb)
                dall = dpool.tile([128, CC, rb, N], bf16, tag="dall")
                for c in range(CC):
                    nc.sync.dma_start(dall[:, c], data_r[:, c, sl, :])
                hyt = mpool.tile([128, rb, N], u8, tag="hye")
                nc.scalar.dma_start(hyt[:], hye[:, sl, :])
                nint = mpool.tile([128, rb, N], bf16, tag="nin")
                nc.scalar.dma_start(nint[:], nin[:, sl, :])

                nc.vector.copy_predicated(
                    dall[:, 0:2],
                    hyt[:, None, :, :].broadcast_to((128, 2, rb, N)),
                    dall[:, 2:4],
                )
                nc.vector.copy_predicated(
                    dall[:, 0],
                    hxt[:, None, :].broadcast_to((128, rb, N)),
                    dall[:, 1],
                )
                nc.vector.tensor_tensor(
                    dall[:, 0], dall[:, 0], nint[:], mult
                )
                nc.scalar.dma_start(out_r[:, sl, :], dall[:, 0])
    nc.finalize()
    return nc


def _get_module():
    if "nc" not in _cache:
        _cache["nc"] = _build_module()
    return _cache["nc"]


def _host_masks(rois):
    """Masks in f32 arithmetic bit-identical to the reference."""
    r = np.asarray(rois, dtype=np.float32)
    x1, y1, x2, y2 = r[:, 0], r[:, 1], r[:, 2], r[:, 3]
    two = np.float32(2.0)
    one = np.float32(1.0)

    xs = np.arange(W, dtype=np.float32)[:, None]  # (W, 1)
    cw = np.maximum(x2 - x1, one)[None, :]  # (1, N)
    fx = np.floor(two * (xs - x1[None, :]) / cw)
    hx = (fx >= 1.0).astype(np.uint8)  # clip(floor, 0, 1) == 1
    inx = (xs >= x1[None, :]) & (xs <= x2[None, :])

    ys = np.arange(H, dtype=np.float32)[:, None]  # (H, 1)
    ch = np.maximum(y2 - y1, one)[None, :]
    fy = np.floor(two * (ys - y1[None, :]) / ch)
    hy = (fy >= 1.0).astype(np.uint8)
    iny = (ys >= y1[None, :]) & (ys <= y2[None, :])

    return hx, inx, hy, iny


def _run(data, rois, trace=False):
    from concourse.bass_utils import run_bass_kernel_spmd

    data = np.asarray(data, dtype=np.float32).astype(BF16)
    hx, inx, hy, iny = _host_masks(rois)

    in_maps = []
    for i in range(NCORES):
        sl = slice(i * WS, (i + 1) * WS)
        wg = _P_W + i * WS  # global column per partition [PV]
        # x-mask: constant per partition
        hxq = np.zeros((P, N), dtype=np.uint8)
        hxq[:PV] = hx[wg]
        # y-mask and combined inside-mask: per (p, r)
        hyq = np.zeros((P, R, N), dtype=np.uint8)
        hyq[:PV] = hy[_PR_H]
        ninq = np.zeros((P, R, N), dtype=BF16)
        ninq[:PV] = (inx[wg][:, None, :] & iny[_PR_H]).astype(BF16)
        # column-major data: [CC, WS, H, N] flattened to [CC, F, N]
        dpad = np.zeros((CC, FP, N), dtype=BF16)
        dpad[:, :F] = (
            data[:, :, sl, :].transpose(0, 2, 1, 3).reshape(CC, F, N)
        )
        in_maps.append(
            {"data": dpad, "hxp": hxq, "hye": hyq, "nin": ninq}
        )

    nc = _get_module()
    last_err = None
    for _attempt in range(2):
        try:
            res = run_bass_kernel_spmd(
                nc, in_maps, core_ids=list(range(NCORES)), trace=trace
            )
            break
        except Exception as e:  # transient NRT device errors: retry once
            last_err = e
    else:
        raise last_err
    full = np.concatenate(
        [
            r["out"][:F].reshape(WS, H, N).transpose(1, 0, 2)
            for r in res.results
        ],
        axis=1,
    )
    return np.asarray(full).astype(np.float32), res


def kernel(data, rois):
    out, _ = _run(data, rois, trace=False)
    return out
